# revision 8
# baseline (speedup 1.0000x reference)
"""Trainium2 Bass kernel for nn_JointLoss (recon MSE + SimCLR contrastive + group distance loss).

v1 strategy (data-parallel over 8 NeuronCores, SPMD via row-rotated proj):
  - Each core owns a 1024-row block of the 8192x8192 similarity matrix;
    np.roll(proj, -c*1024) puts its own rows at local indices 0..1023.
  - PE: bf16 transposes of P (staged through PSUM-as-bf16) build P^T once;
    bf16 matmuls stream sim chunks [128,2048] into ping-pong PSUM halves.
  - exp + row-sum is SPLIT between two engines working concurrently:
      * ScalarE: spline exp in-place on PSUM with accum_out row-sums.
      * VectorE: Schraudolph fast-exp (affine fp32->int16 bit trick, ~1.8%
        per-element, ~3e-4 per-rowsum error) + 16-bit tensor_scalar accum.
  - GpSimd: fp32->bf16 cast of P, recon-MSE (sub + square-accum), distance
    stats, possum (masked diag sums). No partition reduction on device -
    per-partition partials are shipped and reduced on host in float64.
  - All input DMAs on one HWDGE ring in priority order (proj quarters first,
    then xr/rl) so the sim pipeline starts ~3us in.
"""

import sys

if "/opt/trn_rl_repo" not in sys.path:
    sys.path.insert(0, "/opt/trn_rl_repo")

from contextlib import ExitStack

import numpy as np
import ml_dtypes

import concourse.bacc as bacc
import concourse.tile as tile
from concourse import mybir
from concourse.bass_utils import run_bass_kernel_spmd
from concourse.alu_op_type import AluOpType

N = 8192
D = 128
F = 784
NCORES = 8
RPC = N // NCORES  # 1024 rows per core
RT = RPC // 128    # 8 row-tiles per core
NQ = 4             # column quarters (2048 cols each)
TAU = 0.1

f32 = mybir.dt.float32
bf16 = mybir.dt.bfloat16
fp16 = mybir.dt.float16
i16 = mybir.dt.int16

# Schraudolph fp16 fast-exp: bits = round(x*SCH_A + SCH_B); bits as fp16 ~= exp(x/TAU)
SCH_A = 14773.197218702984   # 1024*log2(e)/TAU
SCH_B = 15302.211261493323   # 15360 + tuned bias (zero-mean rowsum error)

# chunk ownership: ACT if (rt+q)%2==0, plus rt==1 q in (0,2) promoted -> 18/14 split
def _act_own(rt, q):
    # ACT is ~1.7x faster per chunk than the DVE fast-exp path -> 21/11 split
    return ((rt + q) % 2 == 0) or (rt % 2 == 1 and q in (1, 3) and rt < 4) or (rt == 5 and q == 0)


def _kernel_body(tc, proj, xr, rl, identbf, mask, rsum_o, possum_o, partials_o):
    nc = tc.nc
    with ExitStack() as ctx:
        consts = ctx.enter_context(tc.tile_pool(name="consts", bufs=1))
        qf = ctx.enter_context(tc.tile_pool(name="qf", bufs=2))
        qb = ctx.enter_context(tc.tile_pool(name="qb", bufs=2))
        big = ctx.enter_context(tc.tile_pool(name="big", bufs=1))
        dpool = ctx.enter_context(tc.tile_pool(name="dpool", bufs=3))
        stats = ctx.enter_context(tc.tile_pool(name="stats", bufs=1))
        psum = ctx.enter_context(tc.tile_pool(name="psum", bufs=1, space="PSUM"))

        ident_sb = consts.tile([128, 128], bf16)
        nc.sync.dma_start(ident_sb, identbf)
        mask_sb = consts.tile([128, 128], f32)
        nc.sync.dma_start(mask_sb, mask)

        pt_bf = big.tile([128, N], bf16)      # P^T in bf16
        xr_sb = big.tile([128, RT, F], f32)
        rl_sb = big.tile([128, RT, F], f32)
        exp16 = big.tile([128, 2, 2048], i16)  # Schraudolph staging (double buffer)
        junk16 = big.tile([128, 2048], fp16)
        sg2 = big.tile([128, 256, 2], f32)
        sgroups = big.tile([128, 256], f32)

        rsum_sb = stats.tile([128, RT, NQ], f32)
        possum_sb = stats.tile([128, RT], f32)
        partials_sb = stats.tile([128, 4], f32)

        pacc = psum.tile([128, 4096], f32)  # all 8 banks, managed manually

        proj_q = proj.rearrange("(q t p) d -> q p t d", q=NQ, p=128)

        # --- input DMAs: proj quarters FIRST (critical path), then xr/rl ---
        qf_tiles = []
        for q in range(NQ):
            t = qf.tile([128, 16, 128], f32, tag="qf")
            nc.sync.dma_start(t, proj_q[q])
            qf_tiles.append(t)
        nc.sync.dma_start(xr_sb, xr.rearrange("(t p) j -> p t j", p=128))
        nc.sync.dma_start(rl_sb, rl.rearrange("(t p) j -> p t j", p=128))

        # --- GpSimd: fp32->bf16 casts of proj quarters (feeds PE transposes) ---
        qb_tiles = []
        for q in range(NQ):
            t = qb.tile([128, 16, 128], bf16, tag="qb")
            nc.vector.tensor_copy(t, qf_tiles[q])
            qb_tiles.append(t)

        # --- main loop over column quarters ---
        for q in range(NQ):
            # PE transposes of this quarter's 16 tiles -> PSUM cols [0,1024) as bf16
            tslab = pacc[:, 0:1024].bitcast(bf16)  # [128, 2048] bf16
            for tl in range(16):
                nc.tensor.transpose(
                    tslab[:, tl * 128 : (tl + 1) * 128], qb_tiles[q][:, tl, :], ident_sb
                )
            # DVE copies the transposed slab out to pt_bf
            nc.vector.tensor_copy(pt_bf[:, q * 2048 : (q + 1) * 2048], tslab)

            for rt in range(RT):
                half = rt % 2
                chunk = pacc[:, half * 2048 : half * 2048 + 2048]
                w = pt_bf[:, rt * 128 : (rt + 1) * 128]
                for c in range(4):
                    nc.tensor.matmul(
                        chunk[:, c * 512 : (c + 1) * 512],
                        w,
                        pt_bf[:, q * 2048 + c * 512 : q * 2048 + (c + 1) * 512],
                        start=True,
                        stop=True,
                    )
                if q == 0:
                    # exact exp of the positive (diagonal) block -> possum path
                    diag_sb = dpool.tile([128, 128], f32, tag="diag")
                    nc.scalar.activation(
                        diag_sb,
                        chunk[:, rt * 128 : rt * 128 + 128],
                        mybir.ActivationFunctionType.Exp,
                        scale=1.0 / TAU,
                    )
                    dm = dpool.tile([128, 128], bf16, tag="dm")
                    nc.gpsimd.tensor_tensor(dm, diag_sb, mask_sb, AluOpType.mult)
                    nc.vector.tensor_scalar(
                        dm, dm, 1.0, 0.0, AluOpType.mult, AluOpType.add,
                        accum_out=possum_sb[:, rt : rt + 1],
                    )
                if _act_own(rt, q):
                    # ScalarE: spline exp in-place + accumulated row-sum
                    nc.scalar.activation(
                        chunk,
                        chunk,
                        mybir.ActivationFunctionType.Exp,
                        scale=1.0 / TAU,
                        accum_out=rsum_sb[:, rt, q : q + 1],
                    )
                else:
                    # VectorE: Schraudolph affine -> int16, then 16-bit accum pass
                    st = exp16[:, q % 2, :]
                    nc.vector.tensor_scalar(
                        st, chunk, SCH_A, SCH_B, AluOpType.mult, AluOpType.add
                    )
                    nc.vector.tensor_scalar(
                        junk16,
                        st.bitcast(fp16),
                        1.0,
                        0.0,
                        AluOpType.mult,
                        AluOpType.add,
                        accum_out=rsum_sb[:, rt, q : q + 1],
                    )

        # --- GpSimd: recon MSE and distance-loss stats (after xr/rl arrive) ---
        # GpSimd does the elementwise work (bf16 out); DVE accumulates at 4x.
        diffb = big.tile([128, RT, F], bf16)
        nc.gpsimd.tensor_tensor(diffb, xr_sb, rl_sb, AluOpType.subtract)
        nc.gpsimd.tensor_tensor(diffb, diffb, diffb, AluOpType.mult)
        nc.vector.tensor_scalar(
            diffb, diffb, 1.0, 0.0, AluOpType.mult, AluOpType.add,
            accum_out=partials_sb[:, 0:1],
        )
        pt4 = pt_bf[:, 0:RPC].rearrange("p (g s) -> p g s", s=4)
        nc.gpsimd.tensor_tensor(sg2, pt4[:, :, 0::2], pt4[:, :, 1::2], AluOpType.add)
        nc.gpsimd.tensor_tensor(sgroups, sg2[:, :, 0], sg2[:, :, 1], AluOpType.add)
        nc.gpsimd.tensor_tensor(sgroups, sgroups, sgroups, AluOpType.mult)
        nc.vector.tensor_scalar(
            sgroups, sgroups, 1.0, 0.0, AluOpType.mult, AluOpType.add,
            accum_out=partials_sb[:, 2:3],
        )
        pown = pt_bf[:, 0:RPC]
        nc.gpsimd.tensor_tensor(pown, pown, pown, AluOpType.mult)
        nc.vector.tensor_scalar(
            pown, pown, 1.0, 0.0, AluOpType.mult, AluOpType.add,
            accum_out=partials_sb[:, 1:2],
        )
        nc.gpsimd.memset(partials_sb[:, 3:4], 0.0)

        nc.sync.dma_start(rsum_o, rsum_sb.rearrange("p t q -> p (t q)"))
        nc.sync.dma_start(possum_o, possum_sb)
        nc.sync.dma_start(partials_o, partials_sb)


def _build():
    nc = bacc.Bacc("TRN2", target_bir_lowering=False, debug=False, num_devices=NCORES)
    proj = nc.dram_tensor("proj", [N, D], f32, kind="ExternalInput").ap()
    xr = nc.dram_tensor("xr", [RPC, F], f32, kind="ExternalInput").ap()
    rl = nc.dram_tensor("rl", [RPC, F], f32, kind="ExternalInput").ap()
    identbf = nc.dram_tensor("identbf", [128, 128], bf16, kind="ExternalInput").ap()
    mask = nc.dram_tensor("mask", [128, 128], f32, kind="ExternalInput").ap()
    rsum_o = nc.dram_tensor("rsum_o", [128, RT * NQ], f32, kind="ExternalOutput").ap()
    possum_o = nc.dram_tensor("possum_o", [128, RT], f32, kind="ExternalOutput").ap()
    partials_o = nc.dram_tensor("partials_o", [128, 4], f32, kind="ExternalOutput").ap()

    with tile.TileContext(nc) as tc:
        _kernel_body(tc, proj, xr, rl, identbf, mask, rsum_o, possum_o, partials_o)
    nc.compile()
    return nc


_NC_CACHE = None


def _get_nc():
    global _NC_CACHE
    if _NC_CACHE is None:
        _NC_CACHE = _build()
    return _NC_CACHE


def _run(projections, xrecon, recon_label, trace=False, **spmd_kwargs):
    nc = _get_nc()
    P = np.ascontiguousarray(np.asarray(projections, dtype=np.float32))
    XR = np.ascontiguousarray(np.asarray(xrecon, dtype=np.float32))
    RL = np.ascontiguousarray(np.asarray(recon_label, dtype=np.float32))
    identbf = np.eye(128, dtype=ml_dtypes.bfloat16)
    mask = np.kron(np.eye(32, dtype=np.float32), np.ones((4, 4), dtype=np.float32))
    in_maps = []
    for c in range(NCORES):
        in_maps.append(
            {
                "proj": np.ascontiguousarray(np.roll(P, -c * RPC, axis=0)),
                "xr": np.ascontiguousarray(XR[c * RPC : (c + 1) * RPC]),
                "rl": np.ascontiguousarray(RL[c * RPC : (c + 1) * RPC]),
                "identbf": identbf,
                "mask": mask,
            }
        )
    return run_bass_kernel_spmd(
        nc, in_maps, core_ids=list(range(NCORES)), trace=trace, **spmd_kwargs
    )


def _combine(results):
    rowsum = np.concatenate(
        [
            results[c]["rsum_o"].reshape(128, RT, NQ).sum(-1).T.reshape(-1)
            for c in range(NCORES)
        ]
    ).astype(np.float64)
    possum = np.concatenate(
        [results[c]["possum_o"].T.reshape(-1) for c in range(NCORES)]
    ).astype(np.float64)
    recon_ss = sum(
        float(results[c]["partials_o"][:, 0].astype(np.float64).sum())
        for c in range(NCORES)
    )
    A = sum(
        float(results[c]["partials_o"][:, 1].astype(np.float64).sum())
        for c in range(NCORES)
    )
    B = sum(
        float(results[c]["partials_o"][:, 2].astype(np.float64).sum())
        for c in range(NCORES)
    )
    closs = float(np.mean(np.log(rowsum) - np.log(possum)))
    recon_loss = recon_ss / (N * F)
    dist_loss = (4.0 * A - B) / ((N // 4) * 6 * D)
    loss = closs + recon_loss + dist_loss
    return (
        np.float32(loss),
        np.float32(closs),
        np.float32(recon_loss),
        np.float32(dist_loss),
    )


def kernel(projections, xrecon, recon_label):
    br = _run(projections, xrecon, recon_label)
    return _combine(br.results)


# revision 11
# speedup vs baseline: 1.0832x; 1.0832x over previous
"""Trainium2 Bass kernel for nn_JointLoss (recon MSE + SimCLR contrastive + group distance loss).

v2 strategy - symmetry-halved exp (data-parallel over 8 NeuronCores, SPMD via
row-rotated proj):
  - sim is symmetric: each exp(sim) block (u,v) serves BOTH row-sums of u's
    rows (free-dim reduce via ScalarE accum_out) and row-sums of v's rows
    (column sums via PE ones-matmul: colsum_j of the block = sum_i exp(sim[j,i])).
  - Each core computes blocks (u, u+d) for its 8 own row-tiles u, d=0..32.
    d=32 pairs {u,u+32} would be double-counted chip-wide, so their column
    sums use a per-core 0/1 weight vector (cores 4-7 contribute zero) and
    the host subtracts the d32 row-part on cores 4-7. Exp elements HALVED
    vs the naive row-block scheme: 33.8k/lane instead of 65.5k.
  - Column sums accumulate in PSUM across u (start=False matmul groups) in
    four partition stripes {0,32,64,96} x 1280 cols; one DVE copy at the
    end drains them. Host assembles global row-sums in float64.
  - GpSimd: recon-MSE elementwise + small stats; DVE: casts, slab copies,
    tiny accumulates.
"""

import sys

if "/opt/trn_rl_repo" not in sys.path:
    sys.path.insert(0, "/opt/trn_rl_repo")

from contextlib import ExitStack

import numpy as np
import ml_dtypes

import concourse.bacc as bacc
import concourse.tile as tile
from concourse import mybir
from concourse.bass_utils import run_bass_kernel_spmd
from concourse.alu_op_type import AluOpType

N = 8192
D = 128
F = 784
NCORES = 8
RPC = N // NCORES  # 1024 rows per core
RT = RPC // 128    # 8 row-tiles per core
NQ = 4
TAU = 0.1

f32 = mybir.dt.float32
bf16 = mybir.dt.bfloat16

Exp = mybir.ActivationFunctionType.Exp

ACC0 = 2816  # start col (f32 units) of the colsum accumulator region in PSUM


def _stripe(v):
    return (v - 1) // 10  # v in 1..39 -> stripe 0..3 (partition 32*s)


def _scol(v):
    return ACC0 + ((v - 1) % 10) * 128  # accumulator column for tile v


def _colsum_segments(u):
    """Split v=u+1..u+31 into runs contiguous in both eu-cols and accum-cols,
    max 4 tiles (512 cols) per matmul."""
    segs = []
    v = u + 1
    while v <= u + 31:
        s = _stripe(v)
        run = 1
        while (
            run < 4
            and v + run <= u + 31
            and _stripe(v + run) == s
        ):
            run += 1
        segs.append((v, run, s))
        v += run
    return segs


def _kernel_body(tc, proj, xr, rl, identbf, maskbf, dmask_in,
                 rsum_o, possum_o, partials_o, colp_o):
    nc = tc.nc
    with ExitStack() as ctx:
        consts = ctx.enter_context(tc.tile_pool(name="consts", bufs=1))
        qf = ctx.enter_context(tc.tile_pool(name="qf", bufs=2))
        qb = ctx.enter_context(tc.tile_pool(name="qb", bufs=2))
        big = ctx.enter_context(tc.tile_pool(name="big", bufs=1))
        dpool = ctx.enter_context(tc.tile_pool(name="dpool", bufs=3))
        stats = ctx.enter_context(tc.tile_pool(name="stats", bufs=1))
        psum = ctx.enter_context(tc.tile_pool(name="psum", bufs=1, space="PSUM"))

        ident_sb = consts.tile([128, 128], bf16)
        nc.sync.dma_start(ident_sb, identbf)
        mask_sb = consts.tile([128, 128], bf16)
        nc.sync.dma_start(mask_sb, maskbf)
        dmask_sb = consts.tile([128, 1], bf16)
        nc.sync.dma_start(dmask_sb, dmask_in)
        ones_sb = consts.tile([128, 1], bf16)
        nc.vector.memset(ones_sb, 1.0)

        pt_bf = big.tile([128, N], bf16)       # P^T in bf16 (tiles 0..63)
        xr_sb = big.tile([128, RT, F], f32)
        rl_sb = big.tile([128, RT, F], f32)
        exp_sb = big.tile([128, 2, 33 * 128], bf16)  # exp blocks, 2-deep ring
        colacc_sb = big.tile([128, 1280], f32)
        sg2 = big.tile([128, 256, 2], f32)
        sgroups = big.tile([128, 256], f32)

        rsum_sb = stats.tile([128, RT, 3], f32)   # chunkA, chunkB, d32raw
        possum_sb = stats.tile([128, RT], f32)
        partials_sb = stats.tile([128, 4], f32)

        pacc = psum.tile([128, 4096], f32)  # all 8 banks, managed manually
        # layout: [0,2176) sim chunk region (A=2048, B=2176, reused)
        #         [2816,4096) colsum accumulators (4 partition stripes)

        proj_q = proj.rearrange("(q t p) d -> q p t d", q=NQ, p=128)

        # --- input DMAs: proj quarters first (critical path), then xr/rl ---
        qf_tiles = []
        for q in range(NQ):
            t = qf.tile([128, 16, 128], f32, tag="qf")
            nc.sync.dma_start(t, proj_q[q])
            qf_tiles.append(t)
        nc.sync.dma_start(xr_sb, xr.rearrange("(t p) j -> p t j", p=128))
        nc.sync.dma_start(rl_sb, rl.rearrange("(t p) j -> p t j", p=128))

        # --- phase T: cast + PE transpose + slab copy, per quarter ---
        for q in range(NQ):
            qbt = qb.tile([128, 16, 128], bf16, tag="qb")
            nc.vector.tensor_copy(qbt, qf_tiles[q])
            tslab = pacc[:, 0:1024].bitcast(bf16)  # [128, 2048] bf16
            for tl in range(16):
                nc.tensor.transpose(
                    tslab[:, tl * 128 : (tl + 1) * 128], qbt[:, tl, :], ident_sb
                )
            nc.vector.tensor_copy(pt_bf[:, q * 2048 : (q + 1) * 2048], tslab)

        # --- phase SIM: per own row-tile u, blocks d=0..32 ---
        def colsum_A(u):
            eu = exp_sb[:, u % 2, :]
            for (v, run, s) in [g for g in _colsum_segments(u) if g[0] <= u + 15]:
                run = min(run, u + 15 - v + 1)
                if run <= 0:
                    continue
                nc.tensor.matmul(
                    pacc[32 * s : 32 * s + 1, _scol(v) : _scol(v) + run * 128],
                    ones_sb,
                    eu[:, (v - u) * 128 : (v - u + run) * 128],
                    start=(u == 0), stop=True, skip_group_check=True,
                    tile_position=(0, 32 * s),
                )

        def colsum_B(u):
            eu = exp_sb[:, u % 2, :]
            for (v, run, s) in _colsum_segments(u):
                hi = min(v + run - 1, u + 31)
                lo = max(v, u + 16)
                if lo > hi:
                    continue
                run2 = hi - lo + 1
                nc.tensor.matmul(
                    pacc[32 * s : 32 * s + 1, _scol(lo) : _scol(lo) + run2 * 128],
                    ones_sb,
                    eu[:, (lo - u) * 128 : (lo - u + run2) * 128],
                    start=(u == 0), stop=True, skip_group_check=True,
                    tile_position=(0, 32 * s),
                )
            # d32: masked via per-core 0/1 weight; always the first write of
            # its slot (first contributor of v=u+32 is u itself)
            v32 = u + 32
            s = _stripe(v32)
            nc.tensor.matmul(
                pacc[32 * s : 32 * s + 1, _scol(v32) : _scol(v32) + 128],
                dmask_sb,
                eu[:, 32 * 128 : 33 * 128],
                start=True, stop=True, skip_group_check=True,
                tile_position=(0, 32 * s),
            )

        for u in range(RT):
            eu = exp_sb[:, u % 2, :]
            w = pt_bf[:, u * 128 : (u + 1) * 128]
            # chunk A: d0..15 -> region [0,2048)
            for c in range(4):
                nc.tensor.matmul(
                    pacc[:, c * 512 : (c + 1) * 512],
                    w,
                    pt_bf[:, u * 128 + c * 512 : u * 128 + (c + 1) * 512],
                    start=True, stop=True,
                )
            if u > 0:
                colsum_B(u - 1)
            nc.scalar.activation(
                eu[:, 0:2048], pacc[:, 0:2048], Exp, scale=1.0 / TAU,
                accum_out=rsum_sb[:, u, 0:1],
            )
            # chunk B: d16..32 (2176 cols) -> region [0,2176) after ACT A
            for c in range(4):
                nc.tensor.matmul(
                    pacc[:, c * 512 : (c + 1) * 512],
                    w,
                    pt_bf[:, u * 128 + 2048 + c * 512 : u * 128 + 2048 + (c + 1) * 512],
                    start=True, stop=True,
                )
            nc.tensor.matmul(
                pacc[:, 2048:2176], w,
                pt_bf[:, u * 128 + 4096 : u * 128 + 4224],
                start=True, stop=True,
            )
            nc.scalar.activation(
                eu[:, 2048:4224], pacc[:, 0:2176], Exp, scale=1.0 / TAU,
                accum_out=rsum_sb[:, u, 1:2],
            )
            # d32 raw row-part (host subtracts it on cores 4-7)
            nc.vector.tensor_scalar(
                eu[:, 4096:4224], eu[:, 4096:4224], 1.0, 0.0,
                AluOpType.mult, AluOpType.add,
                accum_out=rsum_sb[:, u, 2:3],
            )
            # possum: masked diag sums (diag block = eu[:, 0:128])
            dm = dpool.tile([128, 128], bf16, tag="dm")
            nc.gpsimd.tensor_tensor(dm, eu[:, 0:128], mask_sb, AluOpType.mult)
            nc.vector.tensor_scalar(
                dm, dm, 1.0, 0.0, AluOpType.mult, AluOpType.add,
                accum_out=possum_sb[:, u : u + 1],
            )
            colsum_A(u)
        colsum_B(RT - 1)

        # drain colsum accumulators: one full-width copy + DMA
        nc.vector.tensor_copy(colacc_sb, pacc[:, ACC0:4096])
        nc.sync.dma_start(colp_o, colacc_sb)

        # --- recon MSE + distance-loss stats ---
        diffb = big.tile([128, RT, F], bf16)
        nc.gpsimd.tensor_tensor(diffb, xr_sb, rl_sb, AluOpType.subtract)
        nc.gpsimd.tensor_tensor(diffb, diffb, diffb, AluOpType.mult)
        nc.vector.tensor_scalar(
            diffb, diffb, 1.0, 0.0, AluOpType.mult, AluOpType.add,
            accum_out=partials_sb[:, 0:1],
        )
        pt4 = pt_bf[:, 0:RPC].rearrange("p (g s) -> p g s", s=4)
        nc.gpsimd.tensor_tensor(sg2, pt4[:, :, 0::2], pt4[:, :, 1::2], AluOpType.add)
        nc.gpsimd.tensor_tensor(sgroups, sg2[:, :, 0], sg2[:, :, 1], AluOpType.add)
        nc.gpsimd.tensor_tensor(sgroups, sgroups, sgroups, AluOpType.mult)
        nc.vector.tensor_scalar(
            sgroups, sgroups, 1.0, 0.0, AluOpType.mult, AluOpType.add,
            accum_out=partials_sb[:, 2:3],
        )
        pown = pt_bf[:, 0:RPC]
        nc.gpsimd.tensor_tensor(pown, pown, pown, AluOpType.mult)
        nc.vector.tensor_scalar(
            pown, pown, 1.0, 0.0, AluOpType.mult, AluOpType.add,
            accum_out=partials_sb[:, 1:2],
        )
        nc.gpsimd.memset(partials_sb[:, 3:4], 0.0)

        nc.sync.dma_start(rsum_o, rsum_sb.rearrange("p t k -> p (t k)"))
        nc.sync.dma_start(possum_o, possum_sb)
        nc.sync.dma_start(partials_o, partials_sb)


def _build():
    nc = bacc.Bacc("TRN2", target_bir_lowering=False, debug=False, num_devices=NCORES)
    proj = nc.dram_tensor("proj", [N, D], f32, kind="ExternalInput").ap()
    xr = nc.dram_tensor("xr", [RPC, F], f32, kind="ExternalInput").ap()
    rl = nc.dram_tensor("rl", [RPC, F], f32, kind="ExternalInput").ap()
    identbf = nc.dram_tensor("identbf", [128, 128], bf16, kind="ExternalInput").ap()
    maskbf = nc.dram_tensor("maskbf", [128, 128], bf16, kind="ExternalInput").ap()
    dmask_in = nc.dram_tensor("dmask_in", [128, 1], bf16, kind="ExternalInput").ap()
    rsum_o = nc.dram_tensor("rsum_o", [128, RT * 3], f32, kind="ExternalOutput").ap()
    possum_o = nc.dram_tensor("possum_o", [128, RT], f32, kind="ExternalOutput").ap()
    partials_o = nc.dram_tensor("partials_o", [128, 4], f32, kind="ExternalOutput").ap()
    colp_o = nc.dram_tensor("colp_o", [128, 1280], f32, kind="ExternalOutput").ap()

    with tile.TileContext(nc) as tc:
        _kernel_body(tc, proj, xr, rl, identbf, maskbf, dmask_in,
                     rsum_o, possum_o, partials_o, colp_o)
    nc.compile()
    return nc


_NC_CACHE = None


def _get_nc():
    global _NC_CACHE
    if _NC_CACHE is None:
        _NC_CACHE = _build()
    return _NC_CACHE


def _run(projections, xrecon, recon_label, trace=False, **spmd_kwargs):
    nc = _get_nc()
    P = np.ascontiguousarray(np.asarray(projections, dtype=np.float32))
    XR = np.ascontiguousarray(np.asarray(xrecon, dtype=np.float32))
    RL = np.ascontiguousarray(np.asarray(recon_label, dtype=np.float32))
    identbf = np.eye(128, dtype=ml_dtypes.bfloat16)
    maskbf = np.kron(
        np.eye(32, dtype=np.float32), np.ones((4, 4), dtype=np.float32)
    ).astype(ml_dtypes.bfloat16)
    in_maps = []
    for c in range(NCORES):
        dmask = np.full((128, 1), 1.0 if c < 4 else 0.0, dtype=ml_dtypes.bfloat16)
        in_maps.append(
            {
                "proj": np.ascontiguousarray(np.roll(P, -c * RPC, axis=0)),
                "xr": np.ascontiguousarray(XR[c * RPC : (c + 1) * RPC]),
                "rl": np.ascontiguousarray(RL[c * RPC : (c + 1) * RPC]),
                "identbf": identbf,
                "maskbf": maskbf,
                "dmask_in": dmask,
            }
        )
    return run_bass_kernel_spmd(
        nc, in_maps, core_ids=list(range(NCORES)), trace=trace, **spmd_kwargs
    )


def _combine(results):
    NT = N // 128  # 64 global row tiles
    rowsum = np.zeros((NT, 128), dtype=np.float64)
    possum = np.zeros((NT, 128), dtype=np.float64)
    recon_ss = 0.0
    A = 0.0
    B = 0.0
    for c in range(NCORES):
        res = results[c]
        rs = res["rsum_o"].reshape(128, RT, 3).astype(np.float64)
        own = rs[:, :, 0] + rs[:, :, 1]
        if c >= 4:
            own = own - rs[:, :, 2]  # cores 4-7 don't own their d32 blocks
        ps = res["possum_o"].astype(np.float64)
        for u in range(RT):
            rowsum[8 * c + u] += own[:, u]
            possum[8 * c + u] += ps[:, u]
        colacc = res["colp_o"].astype(np.float64)  # [128, 1280]
        for v in range(1, 40):
            s = _stripe(v)
            col = (_scol(v) - ACC0)
            vals = colacc[32 * s, col : col + 128]
            rowsum[(8 * c + v) % NT] += vals
        recon_ss += float(res["partials_o"][:, 0].astype(np.float64).sum())
        A += float(res["partials_o"][:, 1].astype(np.float64).sum())
        B += float(res["partials_o"][:, 2].astype(np.float64).sum())
    rowsum = rowsum.reshape(-1)
    possum = possum.reshape(-1)
    closs = float(np.mean(np.log(rowsum) - np.log(possum)))
    recon_loss = recon_ss / (N * F)
    dist_loss = (4.0 * A - B) / ((N // 4) * 6 * D)
    loss = closs + recon_loss + dist_loss
    return (
        np.float32(loss),
        np.float32(closs),
        np.float32(recon_loss),
        np.float32(dist_loss),
    )


def kernel(projections, xrecon, recon_label):
    br = _run(projections, xrecon, recon_label)
    return _combine(br.results)


# revision 12
# speedup vs baseline: 1.1067x; 1.0217x over previous
"""Trainium2 Bass kernel for nn_JointLoss (recon MSE + SimCLR contrastive + group distance loss).

v2 strategy - symmetry-halved exp (data-parallel over 8 NeuronCores, SPMD via
row-rotated proj):
  - sim is symmetric: each exp(sim) block (u,v) serves BOTH row-sums of u's
    rows (free-dim reduce via ScalarE accum_out) and row-sums of v's rows
    (column sums via PE ones-matmul: colsum_j of the block = sum_i exp(sim[j,i])).
  - Each core computes blocks (u, u+d) for its 8 own row-tiles u, d=0..32.
    d=32 pairs {u,u+32} would be double-counted chip-wide, so their column
    sums use a per-core 0/1 weight vector (cores 4-7 contribute zero) and
    the host subtracts the d32 row-part on cores 4-7. Exp elements HALVED
    vs the naive row-block scheme: 33.8k/lane instead of 65.5k.
  - Column sums accumulate in PSUM across u (start=False matmul groups) in
    four partition stripes {0,32,64,96} x 1280 cols; one DVE copy at the
    end drains them. Host assembles global row-sums in float64.
  - GpSimd: recon-MSE elementwise + small stats; DVE: casts, slab copies,
    tiny accumulates.
"""

import sys

if "/opt/trn_rl_repo" not in sys.path:
    sys.path.insert(0, "/opt/trn_rl_repo")

from contextlib import ExitStack

import numpy as np
import ml_dtypes

import concourse.bacc as bacc
import concourse.tile as tile
from concourse import mybir
from concourse.bass_utils import run_bass_kernel_spmd
from concourse.alu_op_type import AluOpType

N = 8192
D = 128
F = 784
NCORES = 8
RPC = N // NCORES  # 1024 rows per core
RT = RPC // 128    # 8 row-tiles per core
NQ = 4
TAU = 0.1

f32 = mybir.dt.float32
bf16 = mybir.dt.bfloat16

Exp = mybir.ActivationFunctionType.Exp

ACC0 = 2816  # start col (f32 units) of the colsum accumulator region in PSUM


def _stripe(v):
    return (v - 1) // 10  # v in 1..39 -> stripe 0..3 (partition 32*s)


def _scol(v):
    return ACC0 + ((v - 1) % 10) * 128  # accumulator column for tile v


def _colsum_segments(u):
    """Split v=u+1..u+31 into runs contiguous in both eu-cols and accum-cols,
    max 4 tiles (512 cols) per matmul."""
    segs = []
    v = u + 1
    while v <= u + 31:
        s = _stripe(v)
        run = 1
        while (
            run < 4
            and v + run <= u + 31
            and _stripe(v + run) == s
        ):
            run += 1
        segs.append((v, run, s))
        v += run
    return segs


def _kernel_body(tc, proj, xr, rl, identbf, maskbf, dmask_in,
                 rsum_o, possum_o, partials_o, colp_o):
    nc = tc.nc
    with ExitStack() as ctx:
        consts = ctx.enter_context(tc.tile_pool(name="consts", bufs=1))
        qf = ctx.enter_context(tc.tile_pool(name="qf", bufs=2))
        qb = ctx.enter_context(tc.tile_pool(name="qb", bufs=2))
        big = ctx.enter_context(tc.tile_pool(name="big", bufs=1))
        dpool = ctx.enter_context(tc.tile_pool(name="dpool", bufs=3))
        stats = ctx.enter_context(tc.tile_pool(name="stats", bufs=1))
        psum = ctx.enter_context(tc.tile_pool(name="psum", bufs=1, space="PSUM"))

        ident_sb = consts.tile([128, 128], bf16)
        nc.sync.dma_start(ident_sb, identbf)
        mask_sb = consts.tile([128, 128], bf16)
        nc.sync.dma_start(mask_sb, maskbf)
        dmask_sb = consts.tile([128, 1], bf16)
        nc.sync.dma_start(dmask_sb, dmask_in)
        ones_sb = consts.tile([128, 1], bf16)
        nc.vector.memset(ones_sb, 1.0)
        zeros_sb = consts.tile([128, 1], bf16)
        nc.vector.memset(zeros_sb, 0.0)

        pt_bf = big.tile([128, N], bf16)       # P^T in bf16 (tiles 0..63)
        xr_sb = big.tile([128, RT, F], f32)
        rl_sb = big.tile([128, RT, F], f32)
        exp_sb = big.tile([128, 2, 33 * 128], bf16)  # exp blocks, 2-deep ring
        colacc_sb = big.tile([128, 1280], f32)
        sg2 = big.tile([128, 256, 2], f32)
        sgroups = big.tile([128, 256], f32)

        rsum_sb = stats.tile([128, RT, 3], f32)   # chunkA, chunkB, d32raw
        possum_sb = stats.tile([128, RT], f32)
        partials_sb = stats.tile([128, 4], f32)

        pacc = psum.tile([128, 4096], f32)  # all 8 banks, managed manually
        # layout: [0,2176) sim chunk region (A=2048, B=2176, reused)
        #         [2816,4096) colsum accumulators (4 partition stripes)

        proj_q = proj.rearrange("(q t p) d -> q p t d", q=NQ, p=128)

        # --- input DMAs: proj quarters first (critical path), then xr/rl ---
        qf_tiles = []
        for q in range(NQ):
            t = qf.tile([128, 16, 128], f32, tag="qf")
            nc.sync.dma_start(t, proj_q[q])
            qf_tiles.append(t)
        nc.sync.dma_start(xr_sb, xr.rearrange("(t p) j -> p t j", p=128))
        nc.sync.dma_start(rl_sb, rl.rearrange("(t p) j -> p t j", p=128))

        # zero-init colsum accumulators: start=True zero-weight matmuls so
        # all has_written clears happen before any accumulation
        for st in range(4):
            for b in range(10):
                nc.tensor.matmul(
                    pacc[32 * st : 32 * st + 1, ACC0 + b * 128 : ACC0 + (b + 1) * 128],
                    zeros_sb, ident_sb,
                    start=True, stop=True, skip_group_check=True,
                    tile_position=(0, 32 * st),
                )

        # --- phase T: cast + PE transpose + slab copy, per quarter ---
        for q in range(NQ):
            qbt = qb.tile([128, 16, 128], bf16, tag="qb")
            nc.vector.tensor_copy(qbt, qf_tiles[q])
            tslab = pacc[:, 0:1024].bitcast(bf16)  # [128, 2048] bf16
            for tl in range(16):
                nc.tensor.transpose(
                    tslab[:, tl * 128 : (tl + 1) * 128], qbt[:, tl, :], ident_sb
                )
            nc.vector.tensor_copy(pt_bf[:, q * 2048 : (q + 1) * 2048], tslab)

        # --- phase SIM: per own row-tile u, blocks d=0..32 ---
        def colsum_A(u):
            eu = exp_sb[:, u % 2, :]
            for (v, run, s) in [g for g in _colsum_segments(u) if g[0] <= u + 15]:
                run = min(run, u + 15 - v + 1)
                if run <= 0:
                    continue
                nc.tensor.matmul(
                    pacc[32 * s : 32 * s + 1, _scol(v) : _scol(v) + run * 128],
                    ones_sb,
                    eu[:, (v - u) * 128 : (v - u + run) * 128],
                    start=False, stop=True, skip_group_check=True,
                    tile_position=(0, 32 * s),
                )

        def colsum_B(u):
            eu = exp_sb[:, u % 2, :]
            for (v, run, s) in _colsum_segments(u):
                hi = min(v + run - 1, u + 31)
                lo = max(v, u + 16)
                if lo > hi:
                    continue
                run2 = hi - lo + 1
                nc.tensor.matmul(
                    pacc[32 * s : 32 * s + 1, _scol(lo) : _scol(lo) + run2 * 128],
                    ones_sb,
                    eu[:, (lo - u) * 128 : (lo - u + run2) * 128],
                    start=False, stop=True, skip_group_check=True,
                    tile_position=(0, 32 * s),
                )
            # d32: masked via per-core 0/1 weight; always the first write of
            # its slot (first contributor of v=u+32 is u itself)
            v32 = u + 32
            s = _stripe(v32)
            nc.tensor.matmul(
                pacc[32 * s : 32 * s + 1, _scol(v32) : _scol(v32) + 128],
                dmask_sb,
                eu[:, 32 * 128 : 33 * 128],
                start=False, stop=True, skip_group_check=True,
                tile_position=(0, 32 * s),
            )

        for u in range(RT):
            eu = exp_sb[:, u % 2, :]
            w = pt_bf[:, u * 128 : (u + 1) * 128]
            # chunk A: d0..15 -> region [0,2048)
            for c in range(4):
                nc.tensor.matmul(
                    pacc[:, c * 512 : (c + 1) * 512],
                    w,
                    pt_bf[:, u * 128 + c * 512 : u * 128 + (c + 1) * 512],
                    start=True, stop=True,
                )
            if u > 0:
                colsum_B(u - 1)
            nc.scalar.activation(
                eu[:, 0:2048], pacc[:, 0:2048], Exp, scale=1.0 / TAU,
                accum_out=rsum_sb[:, u, 0:1],
            )
            # chunk B: d16..32 (2176 cols) -> region [0,2176) after ACT A
            for c in range(4):
                nc.tensor.matmul(
                    pacc[:, c * 512 : (c + 1) * 512],
                    w,
                    pt_bf[:, u * 128 + 2048 + c * 512 : u * 128 + 2048 + (c + 1) * 512],
                    start=True, stop=True,
                )
            nc.tensor.matmul(
                pacc[:, 2048:2176], w,
                pt_bf[:, u * 128 + 4096 : u * 128 + 4224],
                start=True, stop=True,
            )
            nc.scalar.activation(
                eu[:, 2048:4224], pacc[:, 0:2176], Exp, scale=1.0 / TAU,
                accum_out=rsum_sb[:, u, 1:2],
            )
            # d32 raw row-part (host subtracts it on cores 4-7)
            nc.vector.tensor_scalar(
                eu[:, 4096:4224], eu[:, 4096:4224], 1.0, 0.0,
                AluOpType.mult, AluOpType.add,
                accum_out=rsum_sb[:, u, 2:3],
            )
            # possum: masked diag sums (diag block = eu[:, 0:128])
            dm = dpool.tile([128, 128], bf16, tag="dm")
            nc.gpsimd.tensor_tensor(dm, eu[:, 0:128], mask_sb, AluOpType.mult)
            nc.vector.tensor_scalar(
                dm, dm, 1.0, 0.0, AluOpType.mult, AluOpType.add,
                accum_out=possum_sb[:, u : u + 1],
            )
            colsum_A(u)
        colsum_B(RT - 1)

        # drain colsum accumulators: one full-width copy + DMA
        nc.vector.tensor_copy(colacc_sb, pacc[:, ACC0:4096])
        nc.sync.dma_start(colp_o, colacc_sb)

        # --- recon MSE + distance-loss stats ---
        diffb = big.tile([128, RT, F], bf16)
        nc.gpsimd.tensor_tensor(diffb, xr_sb, rl_sb, AluOpType.subtract)
        nc.gpsimd.tensor_tensor(diffb, diffb, diffb, AluOpType.mult)
        nc.vector.tensor_scalar(
            diffb, diffb, 1.0, 0.0, AluOpType.mult, AluOpType.add,
            accum_out=partials_sb[:, 0:1],
        )
        pt4 = pt_bf[:, 0:RPC].rearrange("p (g s) -> p g s", s=4)
        nc.gpsimd.tensor_tensor(sg2, pt4[:, :, 0::2], pt4[:, :, 1::2], AluOpType.add)
        nc.gpsimd.tensor_tensor(sgroups, sg2[:, :, 0], sg2[:, :, 1], AluOpType.add)
        nc.gpsimd.tensor_tensor(sgroups, sgroups, sgroups, AluOpType.mult)
        nc.vector.tensor_scalar(
            sgroups, sgroups, 1.0, 0.0, AluOpType.mult, AluOpType.add,
            accum_out=partials_sb[:, 2:3],
        )
        pown = pt_bf[:, 0:RPC]
        nc.gpsimd.tensor_tensor(pown, pown, pown, AluOpType.mult)
        nc.vector.tensor_scalar(
            pown, pown, 1.0, 0.0, AluOpType.mult, AluOpType.add,
            accum_out=partials_sb[:, 1:2],
        )
        nc.gpsimd.memset(partials_sb[:, 3:4], 0.0)

        nc.sync.dma_start(rsum_o, rsum_sb.rearrange("p t k -> p (t k)"))
        nc.sync.dma_start(possum_o, possum_sb)
        nc.sync.dma_start(partials_o, partials_sb)


def _build():
    nc = bacc.Bacc("TRN2", target_bir_lowering=False, debug=False, num_devices=NCORES)
    proj = nc.dram_tensor("proj", [N, D], f32, kind="ExternalInput").ap()
    xr = nc.dram_tensor("xr", [RPC, F], f32, kind="ExternalInput").ap()
    rl = nc.dram_tensor("rl", [RPC, F], f32, kind="ExternalInput").ap()
    identbf = nc.dram_tensor("identbf", [128, 128], bf16, kind="ExternalInput").ap()
    maskbf = nc.dram_tensor("maskbf", [128, 128], bf16, kind="ExternalInput").ap()
    dmask_in = nc.dram_tensor("dmask_in", [128, 1], bf16, kind="ExternalInput").ap()
    rsum_o = nc.dram_tensor("rsum_o", [128, RT * 3], f32, kind="ExternalOutput").ap()
    possum_o = nc.dram_tensor("possum_o", [128, RT], f32, kind="ExternalOutput").ap()
    partials_o = nc.dram_tensor("partials_o", [128, 4], f32, kind="ExternalOutput").ap()
    colp_o = nc.dram_tensor("colp_o", [128, 1280], f32, kind="ExternalOutput").ap()

    with tile.TileContext(nc) as tc:
        _kernel_body(tc, proj, xr, rl, identbf, maskbf, dmask_in,
                     rsum_o, possum_o, partials_o, colp_o)
    nc.compile()
    return nc


_NC_CACHE = None


def _get_nc():
    global _NC_CACHE
    if _NC_CACHE is None:
        _NC_CACHE = _build()
    return _NC_CACHE


def _run(projections, xrecon, recon_label, trace=False, **spmd_kwargs):
    nc = _get_nc()
    P = np.ascontiguousarray(np.asarray(projections, dtype=np.float32))
    XR = np.ascontiguousarray(np.asarray(xrecon, dtype=np.float32))
    RL = np.ascontiguousarray(np.asarray(recon_label, dtype=np.float32))
    identbf = np.eye(128, dtype=ml_dtypes.bfloat16)
    maskbf = np.kron(
        np.eye(32, dtype=np.float32), np.ones((4, 4), dtype=np.float32)
    ).astype(ml_dtypes.bfloat16)
    in_maps = []
    for c in range(NCORES):
        dmask = np.full((128, 1), 1.0 if c < 4 else 0.0, dtype=ml_dtypes.bfloat16)
        in_maps.append(
            {
                "proj": np.ascontiguousarray(np.roll(P, -c * RPC, axis=0)),
                "xr": np.ascontiguousarray(XR[c * RPC : (c + 1) * RPC]),
                "rl": np.ascontiguousarray(RL[c * RPC : (c + 1) * RPC]),
                "identbf": identbf,
                "maskbf": maskbf,
                "dmask_in": dmask,
            }
        )
    return run_bass_kernel_spmd(
        nc, in_maps, core_ids=list(range(NCORES)), trace=trace, **spmd_kwargs
    )


def _combine(results):
    NT = N // 128  # 64 global row tiles
    rowsum = np.zeros((NT, 128), dtype=np.float64)
    possum = np.zeros((NT, 128), dtype=np.float64)
    recon_ss = 0.0
    A = 0.0
    B = 0.0
    for c in range(NCORES):
        res = results[c]
        rs = res["rsum_o"].reshape(128, RT, 3).astype(np.float64)
        own = rs[:, :, 0] + rs[:, :, 1]
        if c >= 4:
            own = own - rs[:, :, 2]  # cores 4-7 don't own their d32 blocks
        ps = res["possum_o"].astype(np.float64)
        for u in range(RT):
            rowsum[8 * c + u] += own[:, u]
            possum[8 * c + u] += ps[:, u]
        colacc = res["colp_o"].astype(np.float64)  # [128, 1280]
        for v in range(1, 40):
            s = _stripe(v)
            col = (_scol(v) - ACC0)
            vals = colacc[32 * s, col : col + 128]
            rowsum[(8 * c + v) % NT] += vals
        recon_ss += float(res["partials_o"][:, 0].astype(np.float64).sum())
        A += float(res["partials_o"][:, 1].astype(np.float64).sum())
        B += float(res["partials_o"][:, 2].astype(np.float64).sum())
    rowsum = rowsum.reshape(-1)
    possum = possum.reshape(-1)
    closs = float(np.mean(np.log(rowsum) - np.log(possum)))
    recon_loss = recon_ss / (N * F)
    dist_loss = (4.0 * A - B) / ((N // 4) * 6 * D)
    loss = closs + recon_loss + dist_loss
    return (
        np.float32(loss),
        np.float32(closs),
        np.float32(recon_loss),
        np.float32(dist_loss),
    )


def kernel(projections, xrecon, recon_label):
    br = _run(projections, xrecon, recon_label)
    return _combine(br.results)
